# revision 37
# baseline (speedup 1.0000x reference)
"""Trainium2 Bass kernel for nn_Dihedral (gnn_message_passing, 8 NeuronCores).

kernel(**inputs) -> [256] f32 per-batch dihedral energies.

Design v3 — gather-free streaming, engine-split compute. mapping columns are
consecutive-atom windows (b..b+3), so every per-dihedral quantity except the
batch label is a function of the window start. The host builds, per core, a
batch-sorted per-dihedral stream of 15 bf16 field planes
    [dr1 (3), dr2 (3), dr3 (3), A1..A3, B1'..B3']
(A = -k*cos(th0), B' = sign/2x-folded -k*sin(th0) — the same type-table fold
as v1; the angle-independent C = sum_k term is summed host-side) laid out
tile-contiguously in DRAM, so the device does pure SEQUENTIAL DMA (no
dma_gather, 30B/dihedral vs 256B in v1). Bins are padded to 32-element
blocks along partitions.

Device per core, per [128, F] tile: one dma_start pulls all 15 planes; the
columns are SPLIT between DVE (bf16 2x_1p) and gpsimd, each running the full
torsion chain independently on its slice (no cross-engine dependency
stalls); ACT does squares/sqrts/copies for both slices; PE reduces
32-partition blocks with a [128,4] indicator matmul. The torsion uses the
xy-polynomial form (cos/sin of d*phi expanded in x = n1.n2,
Y = (dr1.n2)|dr2|, u = 1/r — no trig tables). Host: bincount block sums
into 256 bins, sum the 8 per-core partials (the all-reduce of the
sum-sharded output).
"""

import os
import sys
import numpy as np

if "/opt/trn_rl_repo" not in sys.path:
    sys.path.insert(0, "/opt/trn_rl_repo")

import concourse.bass as bass
import concourse.bacc as bacc
import concourse.mybir as mybir
import concourse.tile as tile
from concourse.library_config import standard as std_lib
from concourse.tile_rust import add_dep_helper
import ml_dtypes

P = 128
NCORES = 8
QUANT = 32           # bin padding quantum == PE group size
NGRP = P // QUANT    # 4 partial sums per column
NPLANES = 15
NB = 256


# --------------------------------------------------------------------------
# host-side prep
# --------------------------------------------------------------------------

def build_window_fields(pos, atom_types, thetas, ks):
    """([15, NW] f32 per-window field planes, [NW] f32 C values)."""
    NW = pos.shape[0] - 3
    t3 = thetas.reshape(3, -1).astype(np.float64)
    k3 = ks.reshape(3, -1).astype(np.float64)
    A = (-(k3 * np.cos(t3))).astype(np.float32)      # [3, 390625]
    B = (-(k3 * np.sin(t3))).astype(np.float32)
    C = k3.sum(axis=0).astype(np.float32)
    ty = np.asarray(atom_types).astype(np.int64)
    T4 = ((ty[:NW] * 25 + ty[1:NW + 1]) * 25 + ty[2:NW + 2]) * 25 + ty[3:NW + 3]
    f = np.empty((NPLANES, NW), dtype=np.float32)
    f[0:3] = (pos[1:NW + 1] - pos[0:NW]).T
    f[3:6] = (pos[2:NW + 2] - pos[1:NW + 1]).T
    f[6:9] = (pos[3:NW + 3] - pos[2:NW + 2]).T
    f[9] = A[0, T4]; f[10] = A[1, T4]; f[11] = A[2, T4]
    f[12] = -B[0, T4]; f[13] = -2.0 * B[1, T4]; f[14] = -B[2, T4]
    return f, C[T4]


def tile_widths(max_cols, F, F0):
    """Column widths per tile: a narrow first tile (F0) to shorten pipeline
    fill, then F-wide tiles, last one trimmed to a multiple of 4."""
    widths = [min(F0, max_cols)]
    rem = max_cols - widths[0]
    while rem > 0:
        w = min(F, rem)
        w = ((w + 3) // 4) * 4
        widths.append(w)
        rem = max_cols - sum(widths)
    return widths


def plan_streams(base, batch, n_win, F, F0):
    """Per-core batch-sorted window-index streams, bins padded to QUANT,
    common tile widths across cores."""
    SUBW = (n_win + NCORES - 1) // NCORES
    core_of = base // SUBW

    idx_streams = []
    lab_streams = []
    for c in range(NCORES):
        sel = np.nonzero(core_of == c)[0]        # batch-sorted already
        lab = batch[sel]
        cnt = np.bincount(lab, minlength=NB)
        pcnt = ((cnt + QUANT - 1) // QUANT) * QUANT
        total = int(pcnt.sum())
        idx_out = np.full(total, -1, dtype=np.int64)
        ends = np.cumsum(pcnt)
        starts = ends - pcnt
        within = np.arange(len(sel)) - np.repeat(np.cumsum(cnt) - cnt, cnt)
        idx_out[starts[lab] + within] = base[sel]
        blk_lab = np.repeat(np.arange(NB, dtype=np.int32), pcnt // QUANT)
        idx_streams.append(idx_out)
        lab_streams.append(blk_lab)

    max_cols = max((len(s) + P - 1) // P for s in idx_streams)
    widths = tile_widths(max_cols, F, F0)
    NCOLS = sum(widths)

    streams = np.full((NCORES, NCOLS * P), -1, dtype=np.int64)
    blk_labels = np.full((NCORES, NCOLS * NGRP), -1, dtype=np.int32)
    for c in range(NCORES):
        streams[c, :len(idx_streams[c])] = idx_streams[c]
        blk_labels[c, :len(lab_streams[c])] = lab_streams[c]
    return streams, blk_labels, widths


def build_core_tables(fields, streams, widths, np_dtype):
    """[NCORES, P, NPLANES*NCOLS] plane tables. Global stream index
    s = c*P + p lives at dram[p, k*NCOLS + c] (plane-major rectangle;
    tiles are just column windows)."""
    NCOLS = sum(widths)
    tables = np.empty((NCORES, P, NPLANES * NCOLS), dtype=np_dtype)
    for c in range(NCORES):
        w = streams[c]
        dummy = w < 0
        vals = fields[:, np.where(dummy, 0, w)]          # [15, NCOLS*P]
        if dummy.any():
            vals[9:15][:, dummy] = 0.0                   # V = 0 for padding
        tables[c] = (vals.reshape(NPLANES, NCOLS, P)
                     .transpose(2, 0, 1)
                     .reshape(P, NPLANES * NCOLS).astype(np_dtype))
    return tables


# --------------------------------------------------------------------------
# device program
# --------------------------------------------------------------------------

def build_program(widths, repeat=1, dtype="bf16", gat_bufs=None,
                  tmp_bufs=None):
    if gat_bufs is None:
        gat_bufs = int(os.environ.get("DK_GATB", "3"))
    if tmp_bufs is None:
        tmp_bufs = int(os.environ.get("DK_TMPB", "1"))
    f32 = mybir.dt.float32
    dt = mybir.dt.bfloat16 if dtype == "bf16" else f32
    Alu = mybir.AluOpType
    Act = mybir.ActivationFunctionType
    NCOLS = sum(widths)

    nc = bacc.Bacc("TRN2", target_bir_lowering=False, debug=False)
    tbl = nc.dram_tensor("tbl", [P, NPLANES * NCOLS], dt,
                         kind="ExternalInput").ap()
    out = nc.dram_tensor("out", [NGRP, NCOLS], f32, kind="ExternalOutput").ap()
    tbl3 = tbl.rearrange("p (k c) -> p k c", c=NCOLS)

    with tile.TileContext(nc) as tc:
        with (
            tc.tile_pool(name="gat", bufs=gat_bufs) as gat_pool,
            tc.tile_pool(name="tmp", bufs=tmp_bufs) as tmp_pool,
            tc.tile_pool(name="cst", bufs=1) as cst_pool,
            tc.tile_pool(name="ps", bufs=2, space="PSUM") as ps_pool,
        ):
            lib_inst = nc.gpsimd.load_library(std_lib)

            grp = cst_pool.tile([P, NGRP], dt)
            nc.gpsimd.memset(grp[:], 0.0)
            for g in range(NGRP):
                nc.gpsimd.memset(grp[g * QUANT:(g + 1) * QUANT, g:g + 1], 1.0)

            bs = cst_pool.tile([NGRP, NCOLS], f32)

            def do_side(g, side, n):
                """Full torsion chain for an n-column tile g ([P, 15n],
                plane-major); tensor ops on DVE, squares/sqrt/affine on ACT."""
                def fld(k):
                    return g[:, k * n:(k + 1) * n]

                early = int(os.environ.get("DK_EARLYB", "1"))

                def T(tag, d=dt, bufs=1):
                    return tmp_pool.tile([P, n], d, tag=f"{tag}_{side}",
                                         name=f"{tag}_{side}", bufs=bufs)

                def tt(o, i0, i1, op):
                    nc.vector.tensor_tensor(out=o, in0=i0, in1=i1, op=op)

                ax, ay, az = fld(0), fld(1), fld(2)
                bx, by, bz = fld(3), fld(4), fld(5)
                cx, cy, cz = fld(6), fld(7), fld(8)

                cs = T("cs", bufs=early)
                def cross1(o, m1, m2, m3, m4):
                    tt(o[:], m1, m2, Alu.mult)
                    tt(cs[:], m3, m4, Alu.mult)
                    tt(o[:], o[:], cs[:], Alu.subtract)

                n1x = T("n1x", bufs=early); cross1(n1x, ay, bz, az, by)
                n1y = T("n1y", bufs=early); cross1(n1y, az, bx, ax, bz)
                n1z = T("n1z", bufs=early); cross1(n1z, ax, by, ay, bx)
                n2x = T("n2x", bufs=early); cross1(n2x, by, cz, bz, cy)
                n2y = T("n2y", bufs=early); cross1(n2y, bz, cx, bx, cz)
                n2z = T("n2z", bufs=early); cross1(n2z, bx, cy, by, cx)

                def dot3(o, u0, u1, u2_, v0, v1, v2):
                    st = T("dt", bufs=early)
                    tt(o[:], u0, v0, Alu.mult)
                    tt(st[:], u1, v1, Alu.mult)
                    tt(o[:], o[:], st[:], Alu.add)
                    tt(st[:], u2_, v2, Alu.mult)
                    tt(o[:], o[:], st[:], Alu.add)

                x = T("x"); dot3(x, n1x[:], n1y[:], n1z[:], n2x[:], n2y[:], n2z[:])
                D = T("D"); dot3(D, ax, ay, az, n2x[:], n2y[:], n2z[:])

                bsq = tmp_pool.tile([P, 3 * n], dt, tag=f"bsq_{side}",
                                    name=f"bsq_{side}")
                nc.scalar.square(bsq[:], g[:, 3 * n:6 * n])
                w = T("w")
                tt(w[:], bsq[:, 0:n], bsq[:, n:2 * n], Alu.add)
                tt(w[:], w[:], bsq[:, 2 * n:3 * n], Alu.add)
                L = T("L"); nc.scalar.sqrt(L[:], w[:])
                Y = T("Y"); tt(Y[:], D[:], L[:], Alu.mult)

                xx = T("xx"); nc.scalar.square(xx[:], x[:])
                YY = T("YY"); nc.scalar.square(YY[:], Y[:])
                q = T("q"); tt(q[:], xx[:], YY[:], Alu.add)
                r = T("r", f32); nc.scalar.sqrt(r[:], q[:])
                uf = T("uf", f32)
                nc.vector.reciprocal_approx_fast(uf[:], r[:])
                u = T("u"); nc.scalar.copy(u[:], uf[:])

                # normalized X = cos(phi), Yh = -sin(phi); X^2 + Yh^2 = 1.
                # trig tile layout matches the coef plane order A1 A2 A3 B1
                # B2 B3, so all six Fourier term products collapse into one
                # wide DVE multiply; their sum is absorbed into the PSUM
                # accumulation of the block-sum matmuls.
                Act_ = mybir.ActivationFunctionType
                trig = tmp_pool.tile([P, 6 * n], dt, tag=f"trig_{side}",
                                     name=f"trig_{side}")
                X = trig[:, 0:n]
                c2 = trig[:, n:2 * n]
                c3 = trig[:, 2 * n:3 * n]
                Yh = trig[:, 3 * n:4 * n]
                s2 = trig[:, 4 * n:5 * n]
                s3 = trig[:, 5 * n:6 * n]
                tt(X, x[:], u[:], Alu.mult)
                tt(Yh, Y[:], u[:], Alu.mult)
                YY1 = T("YY1"); nc.scalar.square(YY1[:], Yh)
                nc.scalar.activation(c2, YY1[:], Act_.Copy, bias=1.0,
                                     scale=-2.0)
                c3a = T("c3a")
                nc.scalar.activation(c3a[:], YY1[:], Act_.Copy, bias=1.0,
                                     scale=-4.0)
                s3a = T("s3a")
                nc.scalar.activation(s3a[:], YY1[:], Act_.Copy, bias=3.0,
                                     scale=-4.0)
                tt(s2, X, Yh, Alu.mult)
                tt(c3, c3a[:], X, Alu.mult)
                tt(s3, s3a[:], Yh, Alu.mult)

                terms = tmp_pool.tile([P, 6 * n], dt, tag=f"terms_{side}",
                                      name=f"terms_{side}")
                for i in range(6):
                    tt(terms[:, i * n:(i + 1) * n], fld(9 + i),
                       trig[:, i * n:(i + 1) * n], Alu.mult)
                return terms

            def do_tile(off, Ft):
                g = gat_pool.tile([P, NPLANES * Ft], dt, tag="g")
                g3 = g[:].rearrange("p (k c) -> p k c", c=Ft)
                for k0, k1 in [(0, 6), (6, 9), (9, NPLANES)]:
                    nc.sync.dma_start(out=g3[:, k0:k1, :],
                                      in_=tbl3[:, k0:k1, off:off + Ft])
                terms = do_side(g[:], "d", Ft)
                pt = ps_pool.tile([NGRP, Ft], f32, tag="ps", name="ps")
                for c0 in range(0, Ft, 512):
                    c1 = min(c0 + 512, Ft)
                    for i in range(6):
                        nc.tensor.matmul(out=pt[:, c0:c1], lhsT=grp[:],
                                         rhs=terms[:, i * Ft + c0:i * Ft + c1],
                                         start=(i == 0), stop=(i == 5))
                nc.scalar.activation(bs[:, off:off + Ft], pt[:], Act.Copy)

            def body():
                off = 0
                for Ft in widths:
                    do_tile(off, Ft)
                    off += Ft

            if repeat > 1:
                with tc.For_i(0, repeat, 1):
                    body()
            else:
                body()

            nc.sync.dma_start(out=out[:], in_=bs[:])
    nc.compile()
    return nc


# --------------------------------------------------------------------------
# end to end
# --------------------------------------------------------------------------

def prepare(inputs, F=1024, F0=256, dtype="bf16"):
    pos = np.asarray(inputs["pos"], dtype=np.float32)
    ty = np.asarray(inputs["atom_types"])
    mapping = np.asarray(inputs["mapping"])
    batch = np.asarray(inputs["mapping_batch"]).astype(np.int64)
    base = np.asarray(mapping[0]).astype(np.int64)
    assert all(np.array_equal(np.asarray(mapping[j]), base + j)
               for j in range(1, 4)), "mapping not consecutive; fast path invalid"
    n_win = pos.shape[0] - 3
    fields, Cw = build_window_fields(pos, ty, np.asarray(inputs["thetas"]),
                                     np.asarray(inputs["ks"]))
    streams, blk_labels, widths = plan_streams(base, batch, n_win, F, F0)
    np_dtype = ml_dtypes.bfloat16 if dtype == "bf16" else np.float32
    tables = build_core_tables(fields, streams, widths, np_dtype)
    # angle-independent sum_k term, accumulated host-side
    energy_C = np.bincount(batch, weights=Cw[base].astype(np.float64),
                           minlength=NB)
    plan = dict(widths=widths, blk_labels=blk_labels, dtype=dtype,
                energy_C=energy_C)
    return plan, tables


def finish(plan, outs, n_batch=NB):
    """outs: list per core of [NGRP, NCOLS] block sums -> [256] energy."""
    energy = plan["energy_C"].copy()
    for c in range(NCORES):
        bsums = np.asarray(outs[c])          # [NGRP, NCOLS]
        lab = plan["blk_labels"][c]          # [NCOLS*NGRP], -1 = padding
        vals = bsums.T.ravel()               # block (col, grp) order
        m = lab >= 0
        energy += np.bincount(lab[m], weights=vals[m].astype(np.float64),
                              minlength=n_batch)
    return energy.astype(np.float32)


def _kernel_numpy_fallback(pos, atom_types, mapping, mapping_batch, thetas, ks):
    # Correctness safety net for non-consecutive mappings (never expected).
    p0, p1 = pos[mapping[0]], pos[mapping[1]]
    p2, p3 = pos[mapping[2]], pos[mapping[3]]
    dr1, dr2, dr3 = p1 - p0, p2 - p1, p3 - p2
    n1 = np.cross(dr1, dr2); n2 = np.cross(dr2, dr3)
    m1 = np.cross(n1, dr2 / np.linalg.norm(dr2, axis=-1, keepdims=True))
    x = np.sum(n1 * n2, -1); y = np.sum(m1 * n2, -1)
    theta = np.arctan2(y, x)
    t0, t1, t2, t3 = (atom_types[mapping[j]] for j in range(4))
    th = thetas[:, t0, t1, t2, t3]; kk = ks[:, t0, t1, t2, t3]
    degs = np.arange(1, 4)[:, None]
    V = np.sum(kk * (1.0 - np.cos(degs * theta[None, :] - th)), axis=0)
    return np.bincount(mapping_batch, weights=V.astype(np.float64),
                       minlength=256).astype(np.float32)


def kernel(pos, atom_types, mapping, mapping_batch, thetas, ks):
    from concourse.bass_utils import run_bass_kernel_spmd
    pos = np.asarray(pos, dtype=np.float32)
    atom_types = np.asarray(atom_types)
    mapping = np.asarray(mapping)
    mapping_batch = np.asarray(mapping_batch)
    thetas = np.asarray(thetas, dtype=np.float32)
    ks = np.asarray(ks, dtype=np.float32)

    base = np.asarray(mapping[0]).astype(np.int64)
    if not all(np.array_equal(np.asarray(mapping[j]), base + j)
               for j in range(1, 4)):
        print("kernel.py: non-consecutive mapping; numpy fallback",
              file=sys.stderr)
        return _kernel_numpy_fallback(pos, atom_types, mapping, mapping_batch,
                                      thetas, ks)

    inputs = dict(pos=pos, atom_types=atom_types, mapping=mapping,
                  mapping_batch=mapping_batch, thetas=thetas, ks=ks)
    plan, tables = prepare(inputs, F=1024, F0=256, dtype="bf16")
    nc = build_program(plan["widths"], repeat=1, dtype=plan["dtype"])
    in_maps = [{"tbl": tables[c]} for c in range(NCORES)]
    res = run_bass_kernel_spmd(nc, in_maps, list(range(NCORES)))
    outs = [res.results[c]["out"] for c in range(NCORES)]
    return finish(plan, outs).astype(np.float32)


# revision 40
# speedup vs baseline: 1.1146x; 1.1146x over previous
"""Trainium2 Bass kernel for nn_Dihedral (gnn_message_passing, 8 NeuronCores).

kernel(**inputs) -> [256] f32 per-batch dihedral energies.

Design v4 — gather-free streaming. mapping columns are
consecutive-atom windows (b..b+3), so every per-dihedral quantity except the
batch label is a function of the window start. The host builds, per core, a
batch-sorted per-dihedral stream of 15 bf16 field planes
    [dr1 (3), dr2 (3), dr3 (3), A1..A3, B1'..B3']
(A = -k*cos(th0), B' = sign/2x-folded -k*sin(th0) — the same type-table fold
as v1; the angle-independent C = sum_k term is summed host-side) laid out
tile-contiguously in DRAM, so the device does pure SEQUENTIAL DMA (no
dma_gather, 30B/dihedral vs 256B in v1). Bins are padded to 32-element
blocks along partitions.

Device per core, per [128, Ft] tile (a narrow first tile shortens pipeline
fill): staged dma_starts pull geometry then coefficient planes; DVE (bf16
2x_1p mode) runs the torsion chain, ACT the squares/sqrts/affine steps.
The torsion uses the xy-polynomial form — x = n1.n2, Y = (dr1.n2)|dr2|,
X = x/r, Yh = Y/r, and cos/sin of d*phi as polynomials in X, Yh via
X^2+Yh^2 = 1 (no trig tables). The six Fourier term products' sum is
absorbed into the PSUM accumulation of six [128,4] indicator-matmul block
reductions on PE. Host: bincount block sums into 256 bins, sum the 8
per-core partials (the all-reduce of the sum-sharded output).
"""

import os
import sys
import numpy as np

if "/opt/trn_rl_repo" not in sys.path:
    sys.path.insert(0, "/opt/trn_rl_repo")

import concourse.bass as bass
import concourse.bacc as bacc
import concourse.mybir as mybir
import concourse.tile as tile
from concourse.library_config import standard as std_lib
from concourse.tile_rust import add_dep_helper
import ml_dtypes

P = 128
NCORES = 8
QUANT = 32           # bin padding quantum == PE group size
NGRP = P // QUANT    # 4 partial sums per column
NPLANES = 15
NB = 256


# --------------------------------------------------------------------------
# host-side prep
# --------------------------------------------------------------------------

def build_window_fields(pos, atom_types, thetas, ks):
    """([15, NW] f32 per-window field planes, [NW] f32 C values)."""
    NW = pos.shape[0] - 3
    t3 = thetas.reshape(3, -1).astype(np.float64)
    k3 = ks.reshape(3, -1).astype(np.float64)
    A = (-(k3 * np.cos(t3))).astype(np.float32)      # [3, 390625]
    B = (-(k3 * np.sin(t3))).astype(np.float32)
    C = k3.sum(axis=0).astype(np.float32)
    ty = np.asarray(atom_types).astype(np.int64)
    T4 = ((ty[:NW] * 25 + ty[1:NW + 1]) * 25 + ty[2:NW + 2]) * 25 + ty[3:NW + 3]
    f = np.empty((NPLANES, NW), dtype=np.float32)
    f[0:3] = (pos[1:NW + 1] - pos[0:NW]).T
    f[3:6] = (pos[2:NW + 2] - pos[1:NW + 1]).T
    f[6:9] = (pos[3:NW + 3] - pos[2:NW + 2]).T
    f[9] = A[0, T4]; f[10] = A[1, T4]; f[11] = A[2, T4]
    f[12] = -B[0, T4]; f[13] = -2.0 * B[1, T4]; f[14] = -B[2, T4]
    return f, C[T4]


def tile_widths(max_cols, F, F0):
    """Column widths per tile: a narrow first tile (F0) to shorten pipeline
    fill, then F-wide tiles, last one trimmed to a multiple of 4."""
    widths = [min(F0, max_cols)]
    rem = max_cols - widths[0]
    while rem > 0:
        w = min(F, rem)
        w = ((w + 3) // 4) * 4
        widths.append(w)
        rem = max_cols - sum(widths)
    return widths


def plan_streams(base, batch, n_win, F, F0):
    """Per-core batch-sorted window-index streams, bins padded to QUANT,
    common tile widths across cores."""
    SUBW = (n_win + NCORES - 1) // NCORES
    core_of = base // SUBW

    idx_streams = []
    lab_streams = []
    for c in range(NCORES):
        sel = np.nonzero(core_of == c)[0]        # batch-sorted already
        lab = batch[sel]
        cnt = np.bincount(lab, minlength=NB)
        pcnt = ((cnt + QUANT - 1) // QUANT) * QUANT
        total = int(pcnt.sum())
        idx_out = np.full(total, -1, dtype=np.int64)
        ends = np.cumsum(pcnt)
        starts = ends - pcnt
        within = np.arange(len(sel)) - np.repeat(np.cumsum(cnt) - cnt, cnt)
        idx_out[starts[lab] + within] = base[sel]
        blk_lab = np.repeat(np.arange(NB, dtype=np.int32), pcnt // QUANT)
        idx_streams.append(idx_out)
        lab_streams.append(blk_lab)

    max_cols = max((len(s) + P - 1) // P for s in idx_streams)
    widths = tile_widths(max_cols, F, F0)
    NCOLS = sum(widths)

    streams = np.full((NCORES, NCOLS * P), -1, dtype=np.int64)
    blk_labels = np.full((NCORES, NCOLS * NGRP), -1, dtype=np.int32)
    for c in range(NCORES):
        streams[c, :len(idx_streams[c])] = idx_streams[c]
        blk_labels[c, :len(lab_streams[c])] = lab_streams[c]
    return streams, blk_labels, widths


def build_core_tables(fields, streams, widths, np_dtype):
    """[NCORES, P, NPLANES*NCOLS] plane tables. Global stream index
    s = c*P + p lives at dram[p, k*NCOLS + c] (plane-major rectangle;
    tiles are just column windows)."""
    NCOLS = sum(widths)
    tables = np.empty((NCORES, P, NPLANES * NCOLS), dtype=np_dtype)
    for c in range(NCORES):
        w = streams[c]
        dummy = w < 0
        vals = fields[:, np.where(dummy, 0, w)]          # [15, NCOLS*P]
        if dummy.any():
            vals[9:15][:, dummy] = 0.0                   # V = 0 for padding
        tables[c] = (vals.reshape(NPLANES, NCOLS, P)
                     .transpose(2, 0, 1)
                     .reshape(P, NPLANES * NCOLS).astype(np_dtype))
    return tables


# --------------------------------------------------------------------------
# device program
# --------------------------------------------------------------------------

def build_program(widths, repeat=1, dtype="bf16", gat_bufs=None,
                  tmp_bufs=None):
    if gat_bufs is None:
        gat_bufs = int(os.environ.get("DK_GATB", "3"))
    if tmp_bufs is None:
        tmp_bufs = int(os.environ.get("DK_TMPB", "1"))
    f32 = mybir.dt.float32
    dt = mybir.dt.bfloat16 if dtype == "bf16" else f32
    Alu = mybir.AluOpType
    Act = mybir.ActivationFunctionType
    NCOLS = sum(widths)

    nc = bacc.Bacc("TRN2", target_bir_lowering=False, debug=False)
    tbl = nc.dram_tensor("tbl", [P, NPLANES * NCOLS], dt,
                         kind="ExternalInput").ap()
    out = nc.dram_tensor("out", [NGRP, NCOLS], f32, kind="ExternalOutput").ap()
    tbl3 = tbl.rearrange("p (k c) -> p k c", c=NCOLS)

    with tile.TileContext(nc) as tc:
        with (
            tc.tile_pool(name="gat", bufs=gat_bufs) as gat_pool,
            tc.tile_pool(name="tmp", bufs=tmp_bufs) as tmp_pool,
            tc.tile_pool(name="cst", bufs=1) as cst_pool,
            tc.tile_pool(name="ps", bufs=2, space="PSUM") as ps_pool,
        ):
            lib_inst = nc.gpsimd.load_library(std_lib)

            grp = cst_pool.tile([P, NGRP], dt)
            nc.gpsimd.memset(grp[:], 0.0)
            for g in range(NGRP):
                nc.gpsimd.memset(grp[g * QUANT:(g + 1) * QUANT, g:g + 1], 1.0)

            bs = cst_pool.tile([NGRP, NCOLS], f32)

            def do_side(g, side, n):
                """Full torsion chain for an n-column tile g ([P, 15n],
                plane-major); tensor ops on DVE, squares/sqrt/affine on ACT."""
                def fld(k):
                    return g[:, k * n:(k + 1) * n]

                early = int(os.environ.get("DK_EARLYB", "1"))

                def T(tag, d=dt, bufs=1):
                    return tmp_pool.tile([P, n], d, tag=f"{tag}_{side}",
                                         name=f"{tag}_{side}", bufs=bufs)

                def tt(o, i0, i1, op):
                    nc.vector.tensor_tensor(out=o, in0=i0, in1=i1, op=op)

                ax, ay, az = fld(0), fld(1), fld(2)
                bx, by, bz = fld(3), fld(4), fld(5)
                cx, cy, cz = fld(6), fld(7), fld(8)

                cs = T("cs", bufs=early)
                def cross1(o, m1, m2, m3, m4):
                    tt(o[:], m1, m2, Alu.mult)
                    tt(cs[:], m3, m4, Alu.mult)
                    tt(o[:], o[:], cs[:], Alu.subtract)

                n1x = T("n1x", bufs=early); cross1(n1x, ay, bz, az, by)
                n1y = T("n1y", bufs=early); cross1(n1y, az, bx, ax, bz)
                n1z = T("n1z", bufs=early); cross1(n1z, ax, by, ay, bx)
                n2x = T("n2x", bufs=early); cross1(n2x, by, cz, bz, cy)
                n2y = T("n2y", bufs=early); cross1(n2y, bz, cx, bx, cz)
                n2z = T("n2z", bufs=early); cross1(n2z, bx, cy, by, cx)

                def dot3(o, u0, u1, u2_, v0, v1, v2):
                    st = T("dt", bufs=early)
                    tt(o[:], u0, v0, Alu.mult)
                    tt(st[:], u1, v1, Alu.mult)
                    tt(o[:], o[:], st[:], Alu.add)
                    tt(st[:], u2_, v2, Alu.mult)
                    tt(o[:], o[:], st[:], Alu.add)

                x = T("x"); dot3(x, n1x[:], n1y[:], n1z[:], n2x[:], n2y[:], n2z[:])
                D = T("D"); dot3(D, ax, ay, az, n2x[:], n2y[:], n2z[:])

                bsq = tmp_pool.tile([P, 3 * n], dt, tag=f"bsq_{side}",
                                    name=f"bsq_{side}")
                nc.scalar.square(bsq[:], g[:, 3 * n:6 * n])
                w = T("w")
                tt(w[:], bsq[:, 0:n], bsq[:, n:2 * n], Alu.add)
                tt(w[:], w[:], bsq[:, 2 * n:3 * n], Alu.add)
                L = T("L"); nc.scalar.sqrt(L[:], w[:])
                Y = T("Y"); tt(Y[:], D[:], L[:], Alu.mult)

                xx = T("xx"); nc.scalar.square(xx[:], x[:])
                YY = T("YY"); nc.scalar.square(YY[:], Y[:])
                q = T("q"); tt(q[:], xx[:], YY[:], Alu.add)
                r = T("r", f32); nc.scalar.sqrt(r[:], q[:])
                uf = T("uf", f32)
                nc.vector.reciprocal_approx_fast(uf[:], r[:])
                u = T("u"); nc.scalar.copy(u[:], uf[:])

                # normalized X = cos(phi), Yh = -sin(phi); X^2 + Yh^2 = 1.
                # trig tile layout matches the coef plane order A1 A2 A3 B1
                # B2 B3, so all six Fourier term products collapse into one
                # wide DVE multiply; their sum is absorbed into the PSUM
                # accumulation of the block-sum matmuls.
                Act_ = mybir.ActivationFunctionType
                trig = tmp_pool.tile([P, 6 * n], dt, tag=f"trig_{side}",
                                     name=f"trig_{side}")
                X = trig[:, 0:n]
                c2 = trig[:, n:2 * n]
                c3 = trig[:, 2 * n:3 * n]
                Yh = trig[:, 3 * n:4 * n]
                s2 = trig[:, 4 * n:5 * n]
                s3 = trig[:, 5 * n:6 * n]
                tt(X, x[:], u[:], Alu.mult)
                tt(Yh, Y[:], u[:], Alu.mult)
                YY1 = T("YY1"); nc.scalar.square(YY1[:], Yh)
                nc.scalar.activation(c2, YY1[:], Act_.Copy, bias=1.0,
                                     scale=-2.0)
                c3a = T("c3a")
                nc.scalar.activation(c3a[:], YY1[:], Act_.Copy, bias=1.0,
                                     scale=-4.0)
                s3a = T("s3a")
                nc.scalar.activation(s3a[:], YY1[:], Act_.Copy, bias=3.0,
                                     scale=-4.0)
                tt(s2, X, Yh, Alu.mult)
                tt(c3, c3a[:], X, Alu.mult)
                tt(s3, s3a[:], Yh, Alu.mult)

                terms = tmp_pool.tile([P, 6 * n], dt, tag=f"terms_{side}",
                                      name=f"terms_{side}")
                for i in range(6):
                    tt(terms[:, i * n:(i + 1) * n], fld(9 + i),
                       trig[:, i * n:(i + 1) * n], Alu.mult)
                return terms

            def do_tile(off, Ft):
                g = gat_pool.tile([P, NPLANES * Ft], dt, tag="g")
                g3 = g[:].rearrange("p (k c) -> p k c", c=Ft)
                for k0, k1 in [(0, 6), (6, 9), (9, NPLANES)]:
                    nc.sync.dma_start(out=g3[:, k0:k1, :],
                                      in_=tbl3[:, k0:k1, off:off + Ft])
                terms = do_side(g[:], "d", Ft)
                pt = ps_pool.tile([NGRP, Ft], f32, tag="ps", name="ps")
                for c0 in range(0, Ft, 512):
                    c1 = min(c0 + 512, Ft)
                    for i in range(6):
                        nc.tensor.matmul(out=pt[:, c0:c1], lhsT=grp[:],
                                         rhs=terms[:, i * Ft + c0:i * Ft + c1],
                                         start=(i == 0), stop=(i == 5))
                nc.scalar.activation(bs[:, off:off + Ft], pt[:], Act.Copy)

            def body():
                off = 0
                for Ft in widths:
                    do_tile(off, Ft)
                    off += Ft

            if repeat > 1:
                with tc.For_i(0, repeat, 1):
                    body()
            else:
                body()

            nc.sync.dma_start(out=out[:], in_=bs[:])
    nc.compile()
    return nc


# --------------------------------------------------------------------------
# end to end
# --------------------------------------------------------------------------

def prepare(inputs, F=1024, F0=256, dtype="bf16"):
    pos = np.asarray(inputs["pos"], dtype=np.float32)
    ty = np.asarray(inputs["atom_types"])
    mapping = np.asarray(inputs["mapping"])
    batch = np.asarray(inputs["mapping_batch"]).astype(np.int64)
    base = np.asarray(mapping[0]).astype(np.int64)
    assert all(np.array_equal(np.asarray(mapping[j]), base + j)
               for j in range(1, 4)), "mapping not consecutive; fast path invalid"
    if np.any(np.diff(batch) < 0):
        # plan_streams assumes batch-sorted dihedrals; energy is invariant
        # to the within-bin order, so a stable sort is safe.
        order = np.argsort(batch, kind="stable")
        base = base[order]
        batch = batch[order]
    n_win = pos.shape[0] - 3
    fields, Cw = build_window_fields(pos, ty, np.asarray(inputs["thetas"]),
                                     np.asarray(inputs["ks"]))
    streams, blk_labels, widths = plan_streams(base, batch, n_win, F, F0)
    np_dtype = ml_dtypes.bfloat16 if dtype == "bf16" else np.float32
    tables = build_core_tables(fields, streams, widths, np_dtype)
    # angle-independent sum_k term, accumulated host-side
    energy_C = np.bincount(batch, weights=Cw[base].astype(np.float64),
                           minlength=NB)
    plan = dict(widths=widths, blk_labels=blk_labels, dtype=dtype,
                energy_C=energy_C)
    return plan, tables


def finish(plan, outs, n_batch=NB):
    """outs: list per core of [NGRP, NCOLS] block sums -> [256] energy."""
    energy = plan["energy_C"].copy()
    for c in range(NCORES):
        bsums = np.asarray(outs[c])          # [NGRP, NCOLS]
        lab = plan["blk_labels"][c]          # [NCOLS*NGRP], -1 = padding
        vals = bsums.T.ravel()               # block (col, grp) order
        m = lab >= 0
        energy += np.bincount(lab[m], weights=vals[m].astype(np.float64),
                              minlength=n_batch)
    return energy.astype(np.float32)


def _kernel_numpy_fallback(pos, atom_types, mapping, mapping_batch, thetas, ks):
    # Correctness safety net for non-consecutive mappings (never expected).
    p0, p1 = pos[mapping[0]], pos[mapping[1]]
    p2, p3 = pos[mapping[2]], pos[mapping[3]]
    dr1, dr2, dr3 = p1 - p0, p2 - p1, p3 - p2
    n1 = np.cross(dr1, dr2); n2 = np.cross(dr2, dr3)
    m1 = np.cross(n1, dr2 / np.linalg.norm(dr2, axis=-1, keepdims=True))
    x = np.sum(n1 * n2, -1); y = np.sum(m1 * n2, -1)
    theta = np.arctan2(y, x)
    t0, t1, t2, t3 = (atom_types[mapping[j]] for j in range(4))
    th = thetas[:, t0, t1, t2, t3]; kk = ks[:, t0, t1, t2, t3]
    degs = np.arange(1, 4)[:, None]
    V = np.sum(kk * (1.0 - np.cos(degs * theta[None, :] - th)), axis=0)
    return np.bincount(mapping_batch, weights=V.astype(np.float64),
                       minlength=256).astype(np.float32)


def kernel(pos, atom_types, mapping, mapping_batch, thetas, ks):
    from concourse.bass_utils import run_bass_kernel_spmd
    pos = np.asarray(pos, dtype=np.float32)
    atom_types = np.asarray(atom_types)
    mapping = np.asarray(mapping)
    mapping_batch = np.asarray(mapping_batch)
    thetas = np.asarray(thetas, dtype=np.float32)
    ks = np.asarray(ks, dtype=np.float32)

    base = np.asarray(mapping[0]).astype(np.int64)
    if not all(np.array_equal(np.asarray(mapping[j]), base + j)
               for j in range(1, 4)):
        print("kernel.py: non-consecutive mapping; numpy fallback",
              file=sys.stderr)
        return _kernel_numpy_fallback(pos, atom_types, mapping, mapping_batch,
                                      thetas, ks)

    inputs = dict(pos=pos, atom_types=atom_types, mapping=mapping,
                  mapping_batch=mapping_batch, thetas=thetas, ks=ks)
    plan, tables = prepare(inputs, F=1024, F0=256, dtype="bf16")
    nc = build_program(plan["widths"], repeat=1, dtype=plan["dtype"])
    in_maps = [{"tbl": tables[c]} for c in range(NCORES)]
    res = run_bass_kernel_spmd(nc, in_maps, list(range(NCORES)))
    outs = [res.results[c]["out"] for c in range(NCORES)]
    return finish(plan, outs).astype(np.float32)


# revision 46
# speedup vs baseline: 1.1690x; 1.0487x over previous
"""Trainium2 Bass kernel for nn_Dihedral (gnn_message_passing, 8 NeuronCores).

kernel(**inputs) -> [256] f32 per-batch dihedral energies.

Design v4 — gather-free streaming. mapping columns are
consecutive-atom windows (b..b+3), so every per-dihedral quantity except the
batch label is a function of the window start. The host builds, per core, a
batch-sorted per-dihedral stream of 15 bf16 field planes
    [dr1 (3), dr2 (3), dr3 (3), A1..A3, B1'..B3']
(A = -k*cos(th0), B' = sign/2x-folded -k*sin(th0) — the same type-table fold
as v1; the angle-independent C = sum_k term is summed host-side) laid out
tile-contiguously in DRAM, so the device does pure SEQUENTIAL DMA (no
dma_gather, 30B/dihedral vs 256B in v1). Bins are padded to 32-element
blocks along partitions.

Device per core, per [128, Ft] tile (a narrow first tile shortens pipeline
fill): staged dma_starts pull geometry then coefficient planes; DVE (bf16
2x_1p mode) runs the torsion chain, ACT the squares/sqrts/affine steps.
The torsion uses the xy-polynomial form — x = n1.n2, Y = (dr1.n2)|dr2|,
X = x/r, Yh = Y/r, and cos/sin of d*phi as polynomials in X, Yh via
X^2+Yh^2 = 1 (no trig tables). The six Fourier term products' sum is
absorbed into the PSUM accumulation of six [128,4] indicator-matmul block
reductions on PE. Host: bincount block sums into 256 bins, sum the 8
per-core partials (the all-reduce of the sum-sharded output).
"""

import os
import sys
import numpy as np

if "/opt/trn_rl_repo" not in sys.path:
    sys.path.insert(0, "/opt/trn_rl_repo")

import concourse.bass as bass
import concourse.bacc as bacc
import concourse.mybir as mybir
import concourse.tile as tile
from concourse.library_config import standard as std_lib
from concourse.tile_rust import add_dep_helper
import ml_dtypes

P = 128
NCORES = 8
QUANT = 32           # bin padding quantum == PE group size
NGRP = P // QUANT    # 4 partial sums per column
NPLANES = 16         # dr1(3) dr2(3) dr3(3) |dr2|(1) A1-3 B1'-3'
NB = 256


# --------------------------------------------------------------------------
# host-side prep
# --------------------------------------------------------------------------

def build_window_fields(pos, atom_types, thetas, ks):
    """([15, NW] f32 per-window field planes, [NW] f32 C values)."""
    NW = pos.shape[0] - 3
    t3 = thetas.reshape(3, -1).astype(np.float64)
    k3 = ks.reshape(3, -1).astype(np.float64)
    A = (-(k3 * np.cos(t3))).astype(np.float32)      # [3, 390625]
    B = (-(k3 * np.sin(t3))).astype(np.float32)
    C = k3.sum(axis=0).astype(np.float32)
    ty = np.asarray(atom_types).astype(np.int64)
    T4 = ((ty[:NW] * 25 + ty[1:NW + 1]) * 25 + ty[2:NW + 2]) * 25 + ty[3:NW + 3]
    f = np.empty((NPLANES, NW), dtype=np.float32)
    f[0:3] = (pos[1:NW + 1] - pos[0:NW]).T
    f[3:6] = (pos[2:NW + 2] - pos[1:NW + 1]).T
    f[6:9] = (pos[3:NW + 3] - pos[2:NW + 2]).T
    f[9] = np.sqrt(f[3] ** 2 + f[4] ** 2 + f[5] ** 2)          # |dr2|
    f[10] = A[0, T4]; f[11] = A[1, T4]; f[12] = A[2, T4]
    f[13] = -B[0, T4]; f[14] = -2.0 * B[1, T4]; f[15] = -B[2, T4]
    return f, C[T4]


def tile_widths(max_cols, F, F0):
    """Column widths per tile: a narrow first tile (F0) to shorten pipeline
    fill, then F-wide tiles, last one trimmed to a multiple of 4."""
    widths = [min(F0, max_cols)]
    rem = max_cols - widths[0]
    while rem > 0:
        w = min(F, rem)
        w = ((w + 3) // 4) * 4
        widths.append(w)
        rem = max_cols - sum(widths)
    return widths


def plan_streams(base, batch, n_win, F, F0):
    """Per-core batch-sorted window-index streams, bins padded to QUANT,
    common tile widths across cores."""
    SUBW = (n_win + NCORES - 1) // NCORES
    core_of = base // SUBW

    idx_streams = []
    lab_streams = []
    for c in range(NCORES):
        sel = np.nonzero(core_of == c)[0]        # batch-sorted already
        lab = batch[sel]
        cnt = np.bincount(lab, minlength=NB)
        pcnt = ((cnt + QUANT - 1) // QUANT) * QUANT
        total = int(pcnt.sum())
        idx_out = np.full(total, -1, dtype=np.int64)
        ends = np.cumsum(pcnt)
        starts = ends - pcnt
        within = np.arange(len(sel)) - np.repeat(np.cumsum(cnt) - cnt, cnt)
        idx_out[starts[lab] + within] = base[sel]
        blk_lab = np.repeat(np.arange(NB, dtype=np.int32), pcnt // QUANT)
        idx_streams.append(idx_out)
        lab_streams.append(blk_lab)

    max_cols = max((len(s) + P - 1) // P for s in idx_streams)
    widths = tile_widths(max_cols, F, F0)
    NCOLS = sum(widths)

    streams = np.full((NCORES, NCOLS * P), -1, dtype=np.int64)
    blk_labels = np.full((NCORES, NCOLS * NGRP), -1, dtype=np.int32)
    for c in range(NCORES):
        streams[c, :len(idx_streams[c])] = idx_streams[c]
        blk_labels[c, :len(lab_streams[c])] = lab_streams[c]
    return streams, blk_labels, widths


def build_core_tables(fields, streams, widths, np_dtype):
    """[NCORES, P, NPLANES*NCOLS] plane tables. Global stream index
    s = c*P + p lives at dram[p, k*NCOLS + c] (plane-major rectangle;
    tiles are just column windows)."""
    NCOLS = sum(widths)
    tables = np.empty((NCORES, P, NPLANES * NCOLS), dtype=np_dtype)
    for c in range(NCORES):
        w = streams[c]
        dummy = w < 0
        vals = fields[:, np.where(dummy, 0, w)]          # [16, NCOLS*P]
        if dummy.any():
            vals[10:16][:, dummy] = 0.0                  # V = 0 for padding
        tables[c] = (vals.reshape(NPLANES, NCOLS, P)
                     .transpose(2, 0, 1)
                     .reshape(P, NPLANES * NCOLS).astype(np_dtype))
    return tables


# --------------------------------------------------------------------------
# device program
# --------------------------------------------------------------------------

def build_program(widths, repeat=1, dtype="bf16", gat_bufs=None,
                  tmp_bufs=None):
    if gat_bufs is None:
        gat_bufs = int(os.environ.get("DK_GATB", "3"))
    if tmp_bufs is None:
        tmp_bufs = int(os.environ.get("DK_TMPB", "1"))
    f32 = mybir.dt.float32
    dt = mybir.dt.bfloat16 if dtype == "bf16" else f32
    Alu = mybir.AluOpType
    Act = mybir.ActivationFunctionType
    NCOLS = sum(widths)

    nc = bacc.Bacc("TRN2", target_bir_lowering=False, debug=False)
    tbl = nc.dram_tensor("tbl", [P, NPLANES * NCOLS], dt,
                         kind="ExternalInput").ap()
    out = nc.dram_tensor("out", [NGRP, NCOLS], f32, kind="ExternalOutput").ap()
    tbl3 = tbl.rearrange("p (k c) -> p k c", c=NCOLS)

    with tile.TileContext(nc) as tc:
        with (
            tc.tile_pool(name="gat", bufs=gat_bufs) as gat_pool,
            tc.tile_pool(name="tmp", bufs=tmp_bufs) as tmp_pool,
            tc.tile_pool(name="cst", bufs=1) as cst_pool,
            tc.tile_pool(name="ps", bufs=2, space="PSUM") as ps_pool,
        ):
            lib_inst = nc.gpsimd.load_library(std_lib)

            grp = cst_pool.tile([P, NGRP], dt)
            nc.gpsimd.memset(grp[:], 0.0)
            for g in range(NGRP):
                nc.gpsimd.memset(grp[g * QUANT:(g + 1) * QUANT, g:g + 1], 1.0)

            bs = cst_pool.tile([NGRP, NCOLS], f32)

            def do_side(g, side, n):
                """Full torsion chain for an n-column tile g ([P, 15n],
                plane-major); tensor ops on DVE, squares/sqrt/affine on ACT."""
                def fld(k):
                    return g[:, k * n:(k + 1) * n]

                early = int(os.environ.get("DK_EARLYB", "1"))

                def T(tag, d=dt, bufs=1):
                    return tmp_pool.tile([P, n], d, tag=f"{tag}_{side}",
                                         name=f"{tag}_{side}", bufs=bufs)

                def tt(o, i0, i1, op):
                    nc.vector.tensor_tensor(out=o, in0=i0, in1=i1, op=op)

                ax, ay, az = fld(0), fld(1), fld(2)
                bx, by, bz = fld(3), fld(4), fld(5)
                cx, cy, cz = fld(6), fld(7), fld(8)

                cs = T("cs", bufs=early)
                def cross1(o, m1, m2, m3, m4):
                    tt(o[:], m1, m2, Alu.mult)
                    tt(cs[:], m3, m4, Alu.mult)
                    tt(o[:], o[:], cs[:], Alu.subtract)

                n1x = T("n1x", bufs=early); cross1(n1x, ay, bz, az, by)
                n1y = T("n1y", bufs=early); cross1(n1y, az, bx, ax, bz)
                n1z = T("n1z", bufs=early); cross1(n1z, ax, by, ay, bx)
                n2x = T("n2x", bufs=early); cross1(n2x, by, cz, bz, cy)
                n2y = T("n2y", bufs=early); cross1(n2y, bz, cx, bx, cz)
                n2z = T("n2z", bufs=early); cross1(n2z, bx, cy, by, cx)

                def dot3(o, u0, u1, u2_, v0, v1, v2):
                    st = T("dt", bufs=early)
                    tt(o[:], u0, v0, Alu.mult)
                    tt(st[:], u1, v1, Alu.mult)
                    tt(o[:], o[:], st[:], Alu.add)
                    tt(st[:], u2_, v2, Alu.mult)
                    tt(o[:], o[:], st[:], Alu.add)

                x = T("x"); dot3(x, n1x[:], n1y[:], n1z[:], n2x[:], n2y[:], n2z[:])
                D = T("D"); dot3(D, ax, ay, az, n2x[:], n2y[:], n2z[:])

                Y = T("Y"); tt(Y[:], D[:], fld(9), Alu.mult)   # D*|dr2|

                xx = T("xx"); nc.scalar.square(xx[:], x[:])
                YY = T("YY"); nc.scalar.square(YY[:], Y[:])
                q = T("q"); tt(q[:], xx[:], YY[:], Alu.add)
                r = T("r", f32); nc.scalar.sqrt(r[:], q[:])
                uf = T("uf", f32)
                nc.vector.reciprocal_approx_fast(uf[:], r[:])
                u = T("u"); nc.scalar.copy(u[:], uf[:])

                # normalized X = cos(phi), Yh = -sin(phi); X^2 + Yh^2 = 1.
                # trig tile layout matches the coef plane order A1 A2 A3 B1
                # B2 B3, so all six Fourier term products collapse into one
                # wide DVE multiply; their sum is absorbed into the PSUM
                # accumulation of the block-sum matmuls.
                Act_ = mybir.ActivationFunctionType
                trig = tmp_pool.tile([P, 6 * n], dt, tag=f"trig_{side}",
                                     name=f"trig_{side}")
                X = trig[:, 0:n]
                c2 = trig[:, n:2 * n]
                c3 = trig[:, 2 * n:3 * n]
                Yh = trig[:, 3 * n:4 * n]
                s2 = trig[:, 4 * n:5 * n]
                s3 = trig[:, 5 * n:6 * n]
                tt(X, x[:], u[:], Alu.mult)
                tt(Yh, Y[:], u[:], Alu.mult)
                YY1 = T("YY1"); nc.scalar.square(YY1[:], Yh)
                nc.scalar.activation(c2, YY1[:], Act_.Copy, bias=1.0,
                                     scale=-2.0)
                c3a = T("c3a")
                nc.scalar.activation(c3a[:], YY1[:], Act_.Copy, bias=1.0,
                                     scale=-4.0)
                s3a = T("s3a")
                nc.scalar.activation(s3a[:], YY1[:], Act_.Copy, bias=3.0,
                                     scale=-4.0)
                tt(s2, X, Yh, Alu.mult)
                tt(c3, c3a[:], X, Alu.mult)
                tt(s3, s3a[:], Yh, Alu.mult)

                terms = tmp_pool.tile([P, 6 * n], dt, tag=f"terms_{side}",
                                      name=f"terms_{side}")
                for i in range(6):
                    tt(terms[:, i * n:(i + 1) * n], fld(10 + i),
                       trig[:, i * n:(i + 1) * n], Alu.mult)
                return terms

            def do_tile(off, Ft):
                g = gat_pool.tile([P, NPLANES * Ft], dt, tag="g")
                g3 = g[:].rearrange("p (k c) -> p k c", c=Ft)
                for k0, k1 in [(0, 6), (6, 10), (10, NPLANES)]:
                    nc.sync.dma_start(out=g3[:, k0:k1, :],
                                      in_=tbl3[:, k0:k1, off:off + Ft])
                terms = do_side(g[:], "d", Ft)
                pt = ps_pool.tile([NGRP, Ft], f32, tag="ps", name="ps")
                for c0 in range(0, Ft, 512):
                    c1 = min(c0 + 512, Ft)
                    for i in range(6):
                        nc.tensor.matmul(out=pt[:, c0:c1], lhsT=grp[:],
                                         rhs=terms[:, i * Ft + c0:i * Ft + c1],
                                         start=(i == 0), stop=(i == 5))
                nc.scalar.activation(bs[:, off:off + Ft], pt[:], Act.Copy)

            def body():
                off = 0
                for Ft in widths:
                    do_tile(off, Ft)
                    off += Ft

            if repeat > 1:
                with tc.For_i(0, repeat, 1):
                    body()
            else:
                body()

            nc.sync.dma_start(out=out[:], in_=bs[:])
    nc.compile()
    return nc


# --------------------------------------------------------------------------
# end to end
# --------------------------------------------------------------------------

def prepare(inputs, F=1024, F0=256, dtype="bf16"):
    pos = np.asarray(inputs["pos"], dtype=np.float32)
    ty = np.asarray(inputs["atom_types"])
    mapping = np.asarray(inputs["mapping"])
    batch = np.asarray(inputs["mapping_batch"]).astype(np.int64)
    base = np.asarray(mapping[0]).astype(np.int64)
    assert all(np.array_equal(np.asarray(mapping[j]), base + j)
               for j in range(1, 4)), "mapping not consecutive; fast path invalid"
    if np.any(np.diff(batch) < 0):
        # plan_streams assumes batch-sorted dihedrals; energy is invariant
        # to the within-bin order, so a stable sort is safe.
        order = np.argsort(batch, kind="stable")
        base = base[order]
        batch = batch[order]
    n_win = pos.shape[0] - 3
    fields, Cw = build_window_fields(pos, ty, np.asarray(inputs["thetas"]),
                                     np.asarray(inputs["ks"]))
    streams, blk_labels, widths = plan_streams(base, batch, n_win, F, F0)
    np_dtype = ml_dtypes.bfloat16 if dtype == "bf16" else np.float32
    tables = build_core_tables(fields, streams, widths, np_dtype)
    # angle-independent sum_k term, accumulated host-side
    energy_C = np.bincount(batch, weights=Cw[base].astype(np.float64),
                           minlength=NB)
    plan = dict(widths=widths, blk_labels=blk_labels, dtype=dtype,
                energy_C=energy_C)
    return plan, tables


def finish(plan, outs, n_batch=NB):
    """outs: list per core of [NGRP, NCOLS] block sums -> [256] energy."""
    energy = plan["energy_C"].copy()
    for c in range(NCORES):
        bsums = np.asarray(outs[c])          # [NGRP, NCOLS]
        lab = plan["blk_labels"][c]          # [NCOLS*NGRP], -1 = padding
        vals = bsums.T.ravel()               # block (col, grp) order
        m = lab >= 0
        energy += np.bincount(lab[m], weights=vals[m].astype(np.float64),
                              minlength=n_batch)
    return energy.astype(np.float32)


def _kernel_numpy_fallback(pos, atom_types, mapping, mapping_batch, thetas, ks):
    # Correctness safety net for non-consecutive mappings (never expected).
    p0, p1 = pos[mapping[0]], pos[mapping[1]]
    p2, p3 = pos[mapping[2]], pos[mapping[3]]
    dr1, dr2, dr3 = p1 - p0, p2 - p1, p3 - p2
    n1 = np.cross(dr1, dr2); n2 = np.cross(dr2, dr3)
    m1 = np.cross(n1, dr2 / np.linalg.norm(dr2, axis=-1, keepdims=True))
    x = np.sum(n1 * n2, -1); y = np.sum(m1 * n2, -1)
    theta = np.arctan2(y, x)
    t0, t1, t2, t3 = (atom_types[mapping[j]] for j in range(4))
    th = thetas[:, t0, t1, t2, t3]; kk = ks[:, t0, t1, t2, t3]
    degs = np.arange(1, 4)[:, None]
    V = np.sum(kk * (1.0 - np.cos(degs * theta[None, :] - th)), axis=0)
    return np.bincount(mapping_batch, weights=V.astype(np.float64),
                       minlength=256).astype(np.float32)


def kernel(pos, atom_types, mapping, mapping_batch, thetas, ks):
    from concourse.bass_utils import run_bass_kernel_spmd
    pos = np.asarray(pos, dtype=np.float32)
    atom_types = np.asarray(atom_types)
    mapping = np.asarray(mapping)
    mapping_batch = np.asarray(mapping_batch)
    thetas = np.asarray(thetas, dtype=np.float32)
    ks = np.asarray(ks, dtype=np.float32)

    base = np.asarray(mapping[0]).astype(np.int64)
    if not all(np.array_equal(np.asarray(mapping[j]), base + j)
               for j in range(1, 4)):
        print("kernel.py: non-consecutive mapping; numpy fallback",
              file=sys.stderr)
        return _kernel_numpy_fallback(pos, atom_types, mapping, mapping_batch,
                                      thetas, ks)

    inputs = dict(pos=pos, atom_types=atom_types, mapping=mapping,
                  mapping_batch=mapping_batch, thetas=thetas, ks=ks)
    plan, tables = prepare(inputs, F=1024, F0=256, dtype="bf16")
    nc = build_program(plan["widths"], repeat=1, dtype=plan["dtype"])
    in_maps = [{"tbl": tables[c]} for c in range(NCORES)]
    res = run_bass_kernel_spmd(nc, in_maps, list(range(NCORES)))
    outs = [res.results[c]["out"] for c in range(NCORES)]
    return finish(plan, outs).astype(np.float32)


# revision 48
# speedup vs baseline: 1.2051x; 1.0309x over previous
"""Trainium2 Bass kernel for nn_Dihedral (gnn_message_passing, 8 NeuronCores).

kernel(**inputs) -> [256] f32 per-batch dihedral energies.

Design v4 — gather-free streaming. mapping columns are
consecutive-atom windows (b..b+3), so every per-dihedral quantity except the
batch label is a function of the window start. The host builds, per core, a
batch-sorted per-dihedral stream of 15 bf16 field planes
    [dr1 (3), dr2 (3), dr3 (3), A1..A3, B1'..B3']
(A = -k*cos(th0), B' = sign/2x-folded -k*sin(th0) — the same type-table fold
as v1; the angle-independent C = sum_k term is summed host-side) laid out
tile-contiguously in DRAM, so the device does pure SEQUENTIAL DMA (no
dma_gather, 30B/dihedral vs 256B in v1). Bins are padded to 32-element
blocks along partitions.

Device per core, per [128, Ft] tile (a narrow first tile shortens pipeline
fill): staged dma_starts pull geometry then coefficient planes; DVE (bf16
2x_1p mode) runs the torsion chain, ACT the squares/sqrts/affine steps.
The torsion uses the xy-polynomial form — x = n1.n2, Y = (dr1.n2)|dr2|,
X = x/r, Yh = Y/r, and cos/sin of d*phi as polynomials in X, Yh via
X^2+Yh^2 = 1 (no trig tables). The six Fourier term products' sum is
absorbed into the PSUM accumulation of six [128,4] indicator-matmul block
reductions on PE. Host: bincount block sums into 256 bins, sum the 8
per-core partials (the all-reduce of the sum-sharded output).
"""

import os
import sys
import numpy as np

if "/opt/trn_rl_repo" not in sys.path:
    sys.path.insert(0, "/opt/trn_rl_repo")

import concourse.bass as bass
import concourse.bacc as bacc
import concourse.mybir as mybir
import concourse.tile as tile
from concourse.library_config import standard as std_lib
from concourse.tile_rust import add_dep_helper
import ml_dtypes

P = 128
NCORES = 8
QUANT = 32           # bin padding quantum == PE group size
NGRP = P // QUANT    # 4 partial sums per column
NPLANES = 16         # dr1(3) dr2(3) dr3(3) |dr2|(1) A1-3 B1'-3'
NB = 256


# --------------------------------------------------------------------------
# host-side prep
# --------------------------------------------------------------------------

def build_window_fields(pos, atom_types, thetas, ks):
    """([15, NW] f32 per-window field planes, [NW] f32 C values)."""
    NW = pos.shape[0] - 3
    t3 = thetas.reshape(3, -1).astype(np.float64)
    k3 = ks.reshape(3, -1).astype(np.float64)
    A = (-(k3 * np.cos(t3))).astype(np.float32)      # [3, 390625]
    B = (-(k3 * np.sin(t3))).astype(np.float32)
    C = k3.sum(axis=0).astype(np.float32)
    ty = np.asarray(atom_types).astype(np.int64)
    T4 = ((ty[:NW] * 25 + ty[1:NW + 1]) * 25 + ty[2:NW + 2]) * 25 + ty[3:NW + 3]
    f = np.empty((NPLANES, NW), dtype=np.float32)
    # components in rotated (y, z, x) order so cross-product terms pair
    # into contiguous double-width DVE ops
    rot = [1, 2, 0]
    f[0:3] = (pos[1:NW + 1] - pos[0:NW]).T[rot]
    f[3:6] = (pos[2:NW + 2] - pos[1:NW + 1]).T[rot]
    f[6:9] = (pos[3:NW + 3] - pos[2:NW + 2]).T[rot]
    f[9] = np.sqrt(f[3] ** 2 + f[4] ** 2 + f[5] ** 2)          # |dr2|
    f[10] = A[0, T4]; f[11] = A[1, T4]; f[12] = A[2, T4]
    f[13] = -B[0, T4]; f[14] = -2.0 * B[1, T4]; f[15] = -B[2, T4]
    return f, C[T4]


def tile_widths(max_cols, F, F0):
    """Column widths per tile: a narrow first tile (F0) to shorten pipeline
    fill, then F-wide tiles, last one trimmed to a multiple of 4."""
    widths = [min(F0, max_cols)]
    rem = max_cols - widths[0]
    while rem > 0:
        w = min(F, rem)
        w = ((w + 3) // 4) * 4
        widths.append(w)
        rem = max_cols - sum(widths)
    return widths


def plan_streams(base, batch, n_win, F, F0):
    """Per-core batch-sorted window-index streams, bins padded to QUANT,
    common tile widths across cores."""
    SUBW = (n_win + NCORES - 1) // NCORES
    core_of = base // SUBW

    idx_streams = []
    lab_streams = []
    for c in range(NCORES):
        sel = np.nonzero(core_of == c)[0]        # batch-sorted already
        lab = batch[sel]
        cnt = np.bincount(lab, minlength=NB)
        pcnt = ((cnt + QUANT - 1) // QUANT) * QUANT
        total = int(pcnt.sum())
        idx_out = np.full(total, -1, dtype=np.int64)
        ends = np.cumsum(pcnt)
        starts = ends - pcnt
        within = np.arange(len(sel)) - np.repeat(np.cumsum(cnt) - cnt, cnt)
        idx_out[starts[lab] + within] = base[sel]
        blk_lab = np.repeat(np.arange(NB, dtype=np.int32), pcnt // QUANT)
        idx_streams.append(idx_out)
        lab_streams.append(blk_lab)

    max_cols = max((len(s) + P - 1) // P for s in idx_streams)
    widths = tile_widths(max_cols, F, F0)
    NCOLS = sum(widths)

    streams = np.full((NCORES, NCOLS * P), -1, dtype=np.int64)
    blk_labels = np.full((NCORES, NCOLS * NGRP), -1, dtype=np.int32)
    for c in range(NCORES):
        streams[c, :len(idx_streams[c])] = idx_streams[c]
        blk_labels[c, :len(lab_streams[c])] = lab_streams[c]
    return streams, blk_labels, widths


def build_core_tables(fields, streams, widths, np_dtype):
    """[NCORES, P, NPLANES*NCOLS] plane tables. Global stream index
    s = c*P + p lives at dram[p, k*NCOLS + c] (plane-major rectangle;
    tiles are just column windows)."""
    NCOLS = sum(widths)
    tables = np.empty((NCORES, P, NPLANES * NCOLS), dtype=np_dtype)
    for c in range(NCORES):
        w = streams[c]
        dummy = w < 0
        vals = fields[:, np.where(dummy, 0, w)]          # [16, NCOLS*P]
        if dummy.any():
            vals[10:16][:, dummy] = 0.0                  # V = 0 for padding
        tables[c] = (vals.reshape(NPLANES, NCOLS, P)
                     .transpose(2, 0, 1)
                     .reshape(P, NPLANES * NCOLS).astype(np_dtype))
    return tables


# --------------------------------------------------------------------------
# device program
# --------------------------------------------------------------------------

def build_program(widths, repeat=1, dtype="bf16", gat_bufs=None,
                  tmp_bufs=None):
    if gat_bufs is None:
        gat_bufs = int(os.environ.get("DK_GATB", "3"))
    if tmp_bufs is None:
        tmp_bufs = int(os.environ.get("DK_TMPB", "1"))
    f32 = mybir.dt.float32
    dt = mybir.dt.bfloat16 if dtype == "bf16" else f32
    Alu = mybir.AluOpType
    Act = mybir.ActivationFunctionType
    NCOLS = sum(widths)

    nc = bacc.Bacc("TRN2", target_bir_lowering=False, debug=False)
    tbl = nc.dram_tensor("tbl", [P, NPLANES * NCOLS], dt,
                         kind="ExternalInput").ap()
    out = nc.dram_tensor("out", [NGRP, NCOLS], f32, kind="ExternalOutput").ap()
    tbl3 = tbl.rearrange("p (k c) -> p k c", c=NCOLS)

    with tile.TileContext(nc) as tc:
        with (
            tc.tile_pool(name="gat", bufs=gat_bufs) as gat_pool,
            tc.tile_pool(name="tmp", bufs=tmp_bufs) as tmp_pool,
            tc.tile_pool(name="cst", bufs=1) as cst_pool,
            tc.tile_pool(name="ps", bufs=2, space="PSUM") as ps_pool,
        ):
            lib_inst = nc.gpsimd.load_library(std_lib)

            grp = cst_pool.tile([P, NGRP], dt)
            nc.gpsimd.memset(grp[:], 0.0)
            for g in range(NGRP):
                nc.gpsimd.memset(grp[g * QUANT:(g + 1) * QUANT, g:g + 1], 1.0)

            bs = cst_pool.tile([NGRP, NCOLS], f32)

            def do_side(g, side, n):
                """Full torsion chain for an n-column tile g ([P, 15n],
                plane-major); tensor ops on DVE, squares/sqrt/affine on ACT."""
                def fld(k):
                    return g[:, k * n:(k + 1) * n]

                early = int(os.environ.get("DK_EARLYB", "1"))

                def T(tag, d=dt, bufs=1):
                    return tmp_pool.tile([P, n], d, tag=f"{tag}_{side}",
                                         name=f"{tag}_{side}", bufs=bufs)

                def tt(o, i0, i1, op):
                    nc.vector.tensor_tensor(out=o, in0=i0, in1=i1, op=op)

                # planes 0..8 hold a=[ay|az|ax], b=[by|bz|bx], c=[cy|cz|cx];
                # cross terms pair into contiguous double-width slices:
                # (n1x,n1y) = [ay|az]*[bz|bx] - [az|ax]*[by|bz], etc.
                A01 = g[:, 0:2 * n];      A12 = g[:, n:3 * n]
                ax_, ay_ = g[:, 2 * n:3 * n], g[:, 0:n]
                B01 = g[:, 3 * n:5 * n];  B12 = g[:, 4 * n:6 * n]
                bx_, by_ = g[:, 5 * n:6 * n], g[:, 3 * n:4 * n]
                C01 = g[:, 6 * n:8 * n];  C12 = g[:, 7 * n:9 * n]
                cx_, cy_ = g[:, 8 * n:9 * n], g[:, 6 * n:7 * n]

                cs1 = T("cs1", bufs=early)
                cs2 = tmp_pool.tile([P, 2 * n], dt, tag=f"cs2_{side}",
                                    name=f"cs2_{side}", bufs=early)
                n1 = tmp_pool.tile([P, 3 * n], dt, tag=f"n1_{side}",
                                   name=f"n1_{side}", bufs=early)
                n2 = tmp_pool.tile([P, 3 * n], dt, tag=f"n2_{side}",
                                   name=f"n2_{side}", bufs=early)

                def crossp(o, P01, P12, Q01, Q12, px, py, qx, qy):
                    tt(o[:, 0:2 * n], P01, Q12, Alu.mult)
                    tt(cs2[:], P12, Q01, Alu.mult)
                    tt(o[:, 0:2 * n], o[:, 0:2 * n], cs2[:], Alu.subtract)
                    tt(o[:, 2 * n:3 * n], px, qy, Alu.mult)
                    tt(cs1[:], py, qx, Alu.mult)
                    tt(o[:, 2 * n:3 * n], o[:, 2 * n:3 * n], cs1[:],
                       Alu.subtract)

                crossp(n1, A01, A12, B01, B12, ax_, ay_, bx_, by_)
                crossp(n2, B01, B12, C01, C12, bx_, by_, cx_, cy_)

                # x = n1.n2, D = a.n2 via paired products + half adds
                x = T("x"); D = T("D")
                tt(cs2[:], n1[:, 0:2 * n], n2[:, 0:2 * n], Alu.mult)
                tt(x[:], cs2[:, 0:n], cs2[:, n:2 * n], Alu.add)
                tt(cs1[:], n1[:, 2 * n:3 * n], n2[:, 2 * n:3 * n], Alu.mult)
                tt(x[:], x[:], cs1[:], Alu.add)
                tt(cs2[:], A01, n2[:, n:3 * n], Alu.mult)
                tt(D[:], cs2[:, 0:n], cs2[:, n:2 * n], Alu.add)
                tt(cs1[:], ax_, n2[:, 0:n], Alu.mult)
                tt(D[:], D[:], cs1[:], Alu.add)

                Y = T("Y"); tt(Y[:], D[:], fld(9), Alu.mult)   # D*|dr2|

                xx = T("xx"); nc.scalar.square(xx[:], x[:])
                YY = T("YY"); nc.scalar.square(YY[:], Y[:])
                q = T("q"); tt(q[:], xx[:], YY[:], Alu.add)
                r = T("r", f32); nc.scalar.sqrt(r[:], q[:])
                uf = T("uf", f32)
                nc.vector.reciprocal_approx_fast(uf[:], r[:])
                u = T("u"); nc.scalar.copy(u[:], uf[:])

                # normalized X = cos(phi), Yh = -sin(phi); X^2 + Yh^2 = 1.
                # trig tile layout matches the coef plane order A1 A2 A3 B1
                # B2 B3, so all six Fourier term products collapse into one
                # wide DVE multiply; their sum is absorbed into the PSUM
                # accumulation of the block-sum matmuls.
                Act_ = mybir.ActivationFunctionType
                trig = tmp_pool.tile([P, 6 * n], dt, tag=f"trig_{side}",
                                     name=f"trig_{side}")
                X = trig[:, 0:n]
                c2 = trig[:, n:2 * n]
                c3 = trig[:, 2 * n:3 * n]
                Yh = trig[:, 3 * n:4 * n]
                s2 = trig[:, 4 * n:5 * n]
                s3 = trig[:, 5 * n:6 * n]
                tt(X, x[:], u[:], Alu.mult)
                tt(Yh, Y[:], u[:], Alu.mult)
                YY1 = T("YY1"); nc.scalar.square(YY1[:], Yh)
                nc.scalar.activation(c2, YY1[:], Act_.Copy, bias=1.0,
                                     scale=-2.0)
                c3a = T("c3a")
                nc.scalar.activation(c3a[:], YY1[:], Act_.Copy, bias=1.0,
                                     scale=-4.0)
                s3a = T("s3a")
                nc.scalar.activation(s3a[:], YY1[:], Act_.Copy, bias=3.0,
                                     scale=-4.0)
                tt(s2, X, Yh, Alu.mult)
                tt(c3, c3a[:], X, Alu.mult)
                tt(s3, s3a[:], Yh, Alu.mult)

                terms = tmp_pool.tile([P, 6 * n], dt, tag=f"terms_{side}",
                                      name=f"terms_{side}")
                for i in range(6):
                    tt(terms[:, i * n:(i + 1) * n], fld(10 + i),
                       trig[:, i * n:(i + 1) * n], Alu.mult)
                return terms

            def do_tile(off, Ft):
                g = gat_pool.tile([P, NPLANES * Ft], dt, tag="g")
                g3 = g[:].rearrange("p (k c) -> p k c", c=Ft)
                for k0, k1 in [(0, 6), (6, 10), (10, NPLANES)]:
                    nc.sync.dma_start(out=g3[:, k0:k1, :],
                                      in_=tbl3[:, k0:k1, off:off + Ft])
                terms = do_side(g[:], "d", Ft)
                pt = ps_pool.tile([NGRP, Ft], f32, tag="ps", name="ps")
                for c0 in range(0, Ft, 512):
                    c1 = min(c0 + 512, Ft)
                    for i in range(6):
                        nc.tensor.matmul(out=pt[:, c0:c1], lhsT=grp[:],
                                         rhs=terms[:, i * Ft + c0:i * Ft + c1],
                                         start=(i == 0), stop=(i == 5))
                nc.scalar.activation(bs[:, off:off + Ft], pt[:], Act.Copy)

            def body():
                off = 0
                for Ft in widths:
                    do_tile(off, Ft)
                    off += Ft

            if repeat > 1:
                with tc.For_i(0, repeat, 1):
                    body()
            else:
                body()

            nc.sync.dma_start(out=out[:], in_=bs[:])
    nc.compile()
    return nc


# --------------------------------------------------------------------------
# end to end
# --------------------------------------------------------------------------

def prepare(inputs, F=1024, F0=256, dtype="bf16"):
    pos = np.asarray(inputs["pos"], dtype=np.float32)
    ty = np.asarray(inputs["atom_types"])
    mapping = np.asarray(inputs["mapping"])
    batch = np.asarray(inputs["mapping_batch"]).astype(np.int64)
    base = np.asarray(mapping[0]).astype(np.int64)
    assert all(np.array_equal(np.asarray(mapping[j]), base + j)
               for j in range(1, 4)), "mapping not consecutive; fast path invalid"
    if np.any(np.diff(batch) < 0):
        # plan_streams assumes batch-sorted dihedrals; energy is invariant
        # to the within-bin order, so a stable sort is safe.
        order = np.argsort(batch, kind="stable")
        base = base[order]
        batch = batch[order]
    n_win = pos.shape[0] - 3
    fields, Cw = build_window_fields(pos, ty, np.asarray(inputs["thetas"]),
                                     np.asarray(inputs["ks"]))
    streams, blk_labels, widths = plan_streams(base, batch, n_win, F, F0)
    np_dtype = ml_dtypes.bfloat16 if dtype == "bf16" else np.float32
    tables = build_core_tables(fields, streams, widths, np_dtype)
    # angle-independent sum_k term, accumulated host-side
    energy_C = np.bincount(batch, weights=Cw[base].astype(np.float64),
                           minlength=NB)
    plan = dict(widths=widths, blk_labels=blk_labels, dtype=dtype,
                energy_C=energy_C)
    return plan, tables


def finish(plan, outs, n_batch=NB):
    """outs: list per core of [NGRP, NCOLS] block sums -> [256] energy."""
    energy = plan["energy_C"].copy()
    for c in range(NCORES):
        bsums = np.asarray(outs[c])          # [NGRP, NCOLS]
        lab = plan["blk_labels"][c]          # [NCOLS*NGRP], -1 = padding
        vals = bsums.T.ravel()               # block (col, grp) order
        m = lab >= 0
        energy += np.bincount(lab[m], weights=vals[m].astype(np.float64),
                              minlength=n_batch)
    return energy.astype(np.float32)


def _kernel_numpy_fallback(pos, atom_types, mapping, mapping_batch, thetas, ks):
    # Correctness safety net for non-consecutive mappings (never expected).
    p0, p1 = pos[mapping[0]], pos[mapping[1]]
    p2, p3 = pos[mapping[2]], pos[mapping[3]]
    dr1, dr2, dr3 = p1 - p0, p2 - p1, p3 - p2
    n1 = np.cross(dr1, dr2); n2 = np.cross(dr2, dr3)
    m1 = np.cross(n1, dr2 / np.linalg.norm(dr2, axis=-1, keepdims=True))
    x = np.sum(n1 * n2, -1); y = np.sum(m1 * n2, -1)
    theta = np.arctan2(y, x)
    t0, t1, t2, t3 = (atom_types[mapping[j]] for j in range(4))
    th = thetas[:, t0, t1, t2, t3]; kk = ks[:, t0, t1, t2, t3]
    degs = np.arange(1, 4)[:, None]
    V = np.sum(kk * (1.0 - np.cos(degs * theta[None, :] - th)), axis=0)
    return np.bincount(mapping_batch, weights=V.astype(np.float64),
                       minlength=256).astype(np.float32)


def kernel(pos, atom_types, mapping, mapping_batch, thetas, ks):
    from concourse.bass_utils import run_bass_kernel_spmd
    pos = np.asarray(pos, dtype=np.float32)
    atom_types = np.asarray(atom_types)
    mapping = np.asarray(mapping)
    mapping_batch = np.asarray(mapping_batch)
    thetas = np.asarray(thetas, dtype=np.float32)
    ks = np.asarray(ks, dtype=np.float32)

    base = np.asarray(mapping[0]).astype(np.int64)
    if not all(np.array_equal(np.asarray(mapping[j]), base + j)
               for j in range(1, 4)):
        print("kernel.py: non-consecutive mapping; numpy fallback",
              file=sys.stderr)
        return _kernel_numpy_fallback(pos, atom_types, mapping, mapping_batch,
                                      thetas, ks)

    inputs = dict(pos=pos, atom_types=atom_types, mapping=mapping,
                  mapping_batch=mapping_batch, thetas=thetas, ks=ks)
    plan, tables = prepare(inputs, F=1024, F0=256, dtype="bf16")
    nc = build_program(plan["widths"], repeat=1, dtype=plan["dtype"])
    in_maps = [{"tbl": tables[c]} for c in range(NCORES)]
    res = run_bass_kernel_spmd(nc, in_maps, list(range(NCORES)))
    outs = [res.results[c]["out"] for c in range(NCORES)]
    return finish(plan, outs).astype(np.float32)


# revision 51
# speedup vs baseline: 1.2315x; 1.0219x over previous
"""Trainium2 Bass kernel for nn_Dihedral (gnn_message_passing, 8 NeuronCores).

kernel(**inputs) -> [256] f32 per-batch dihedral energies.

Design v4 — gather-free streaming. mapping columns are
consecutive-atom windows (b..b+3), so every per-dihedral quantity except the
batch label is a function of the window start. The host builds, per core, a
batch-sorted per-dihedral stream of 15 bf16 field planes
    [dr1 (3), dr2 (3), dr3 (3), A1..A3, B1'..B3']
(A = -k*cos(th0), B' = sign/2x-folded -k*sin(th0) — the same type-table fold
as v1; the angle-independent C = sum_k term is summed host-side) laid out
tile-contiguously in DRAM, so the device does pure SEQUENTIAL DMA (no
dma_gather, 30B/dihedral vs 256B in v1). Bins are padded to 32-element
blocks along partitions.

Device per core, per [128, Ft] tile (a narrow first tile shortens pipeline
fill): staged dma_starts pull geometry then coefficient planes; DVE (bf16
2x_1p mode) runs the torsion chain, ACT the squares/sqrts/affine steps.
The torsion uses the xy-polynomial form — x = n1.n2, Y = (dr1.n2)|dr2|,
X = x/r, Yh = Y/r, and cos/sin of d*phi as polynomials in X, Yh via
X^2+Yh^2 = 1 (no trig tables). The six Fourier term products' sum is
absorbed into the PSUM accumulation of six [128,4] indicator-matmul block
reductions on PE. Host: bincount block sums into 256 bins, sum the 8
per-core partials (the all-reduce of the sum-sharded output).
"""

import os
import sys
import numpy as np

if "/opt/trn_rl_repo" not in sys.path:
    sys.path.insert(0, "/opt/trn_rl_repo")

import concourse.bass as bass
import concourse.bacc as bacc
import concourse.mybir as mybir
import concourse.tile as tile
from concourse.library_config import standard as std_lib
from concourse.tile_rust import add_dep_helper
import ml_dtypes

P = 128
NCORES = 8
QUANT = 32           # bin padding quantum == PE group size
NGRP = P // QUANT    # 4 partial sums per column
NPLANES = 16         # dr1(3) dr2(3) dr3(3) |dr2|(1) A1-3 B1'-3'
NB = 256


# --------------------------------------------------------------------------
# host-side prep
# --------------------------------------------------------------------------

def build_window_fields(pos, atom_types, thetas, ks):
    """([15, NW] f32 per-window field planes, [NW] f32 C values)."""
    NW = pos.shape[0] - 3
    t3 = thetas.reshape(3, -1).astype(np.float64)
    k3 = ks.reshape(3, -1).astype(np.float64)
    A = (-(k3 * np.cos(t3))).astype(np.float32)      # [3, 390625]
    B = (-(k3 * np.sin(t3))).astype(np.float32)
    C = k3.sum(axis=0).astype(np.float32)
    ty = np.asarray(atom_types).astype(np.int64)
    T4 = ((ty[:NW] * 25 + ty[1:NW + 1]) * 25 + ty[2:NW + 2]) * 25 + ty[3:NW + 3]
    f = np.empty((NPLANES, NW), dtype=np.float32)
    # components in rotated (y, z, x) order so cross-product terms pair
    # into contiguous double-width DVE ops
    rot = [1, 2, 0]
    f[0:3] = (pos[1:NW + 1] - pos[0:NW]).T[rot]
    f[3:6] = (pos[2:NW + 2] - pos[1:NW + 1]).T[rot]
    f[6:9] = (pos[3:NW + 3] - pos[2:NW + 2]).T[rot]
    f[9] = np.sqrt(f[3] ** 2 + f[4] ** 2 + f[5] ** 2)          # |dr2|
    # coef order [A1 B1' A2 B2' A3 B3'] pairs with trig [X Yh c2 s2 c3 s3]
    f[10] = A[0, T4]; f[11] = -B[0, T4]
    f[12] = A[1, T4]; f[13] = -2.0 * B[1, T4]
    f[14] = A[2, T4]; f[15] = -B[2, T4]
    return f, C[T4]


def tile_widths(max_cols, F, F0):
    """Column widths per tile: a narrow first tile (F0) to shorten pipeline
    fill, then F-wide tiles, last one trimmed to a multiple of 4."""
    widths = [min(F0, max_cols)]
    rem = max_cols - widths[0]
    while rem > 0:
        w = min(F, rem)
        w = ((w + 3) // 4) * 4
        widths.append(w)
        rem = max_cols - sum(widths)
    return widths


def plan_streams(base, batch, n_win, F, F0):
    """Per-core batch-sorted window-index streams, bins padded to QUANT,
    common tile widths across cores."""
    SUBW = (n_win + NCORES - 1) // NCORES
    core_of = base // SUBW

    idx_streams = []
    lab_streams = []
    for c in range(NCORES):
        sel = np.nonzero(core_of == c)[0]        # batch-sorted already
        lab = batch[sel]
        cnt = np.bincount(lab, minlength=NB)
        pcnt = ((cnt + QUANT - 1) // QUANT) * QUANT
        total = int(pcnt.sum())
        idx_out = np.full(total, -1, dtype=np.int64)
        ends = np.cumsum(pcnt)
        starts = ends - pcnt
        within = np.arange(len(sel)) - np.repeat(np.cumsum(cnt) - cnt, cnt)
        idx_out[starts[lab] + within] = base[sel]
        blk_lab = np.repeat(np.arange(NB, dtype=np.int32), pcnt // QUANT)
        idx_streams.append(idx_out)
        lab_streams.append(blk_lab)

    max_cols = max((len(s) + P - 1) // P for s in idx_streams)
    widths = tile_widths(max_cols, F, F0)
    NCOLS = sum(widths)

    streams = np.full((NCORES, NCOLS * P), -1, dtype=np.int64)
    blk_labels = np.full((NCORES, NCOLS * NGRP), -1, dtype=np.int32)
    for c in range(NCORES):
        streams[c, :len(idx_streams[c])] = idx_streams[c]
        blk_labels[c, :len(lab_streams[c])] = lab_streams[c]
    return streams, blk_labels, widths


def build_core_tables(fields, streams, widths, np_dtype):
    """[NCORES, P, NPLANES*NCOLS] plane tables. Global stream index
    s = c*P + p lives at dram[p, k*NCOLS + c] (plane-major rectangle;
    tiles are just column windows)."""
    NCOLS = sum(widths)
    tables = np.empty((NCORES, P, NPLANES * NCOLS), dtype=np_dtype)
    for c in range(NCORES):
        w = streams[c]
        dummy = w < 0
        vals = fields[:, np.where(dummy, 0, w)]          # [16, NCOLS*P]
        if dummy.any():
            vals[10:16][:, dummy] = 0.0                  # V = 0 for padding
        tables[c] = (vals.reshape(NPLANES, NCOLS, P)
                     .transpose(2, 0, 1)
                     .reshape(P, NPLANES * NCOLS).astype(np_dtype))
    return tables


# --------------------------------------------------------------------------
# device program
# --------------------------------------------------------------------------

def build_program(widths, repeat=1, dtype="bf16", gat_bufs=None,
                  tmp_bufs=None):
    if gat_bufs is None:
        gat_bufs = int(os.environ.get("DK_GATB", "3"))
    if tmp_bufs is None:
        tmp_bufs = int(os.environ.get("DK_TMPB", "1"))
    f32 = mybir.dt.float32
    dt = mybir.dt.bfloat16 if dtype == "bf16" else f32
    Alu = mybir.AluOpType
    Act = mybir.ActivationFunctionType
    NCOLS = sum(widths)

    nc = bacc.Bacc("TRN2", target_bir_lowering=False, debug=False)
    tbl = nc.dram_tensor("tbl", [P, NPLANES * NCOLS], dt,
                         kind="ExternalInput").ap()
    out = nc.dram_tensor("out", [NGRP, NCOLS], f32, kind="ExternalOutput").ap()
    tbl3 = tbl.rearrange("p (k c) -> p k c", c=NCOLS)

    with tile.TileContext(nc) as tc:
        with (
            tc.tile_pool(name="gat", bufs=gat_bufs) as gat_pool,
            tc.tile_pool(name="tmp", bufs=tmp_bufs) as tmp_pool,
            tc.tile_pool(name="cst", bufs=1) as cst_pool,
            tc.tile_pool(name="ps", bufs=2, space="PSUM") as ps_pool,
        ):
            lib_inst = nc.gpsimd.load_library(std_lib)

            grp = cst_pool.tile([P, NGRP], dt)
            nc.gpsimd.memset(grp[:], 0.0)
            for g in range(NGRP):
                nc.gpsimd.memset(grp[g * QUANT:(g + 1) * QUANT, g:g + 1], 1.0)

            bs = cst_pool.tile([NGRP, NCOLS], f32)

            def do_side(g, side, n):
                """Full torsion chain for an n-column tile g ([P, 15n],
                plane-major); tensor ops on DVE, squares/sqrt/affine on ACT."""
                def fld(k):
                    return g[:, k * n:(k + 1) * n]

                early = int(os.environ.get("DK_EARLYB", "1"))

                def T(tag, d=dt, bufs=1):
                    return tmp_pool.tile([P, n], d, tag=f"{tag}_{side}",
                                         name=f"{tag}_{side}", bufs=bufs)

                def tt(o, i0, i1, op):
                    nc.vector.tensor_tensor(out=o, in0=i0, in1=i1, op=op)

                # planes 0..8 hold a=[ay|az|ax], b=[by|bz|bx], c=[cy|cz|cx];
                # cross terms pair into contiguous double-width slices:
                # (n1x,n1y) = [ay|az]*[bz|bx] - [az|ax]*[by|bz], etc.
                A01 = g[:, 0:2 * n];      A12 = g[:, n:3 * n]
                ax_, ay_ = g[:, 2 * n:3 * n], g[:, 0:n]
                B01 = g[:, 3 * n:5 * n];  B12 = g[:, 4 * n:6 * n]
                bx_, by_ = g[:, 5 * n:6 * n], g[:, 3 * n:4 * n]
                C01 = g[:, 6 * n:8 * n];  C12 = g[:, 7 * n:9 * n]
                cx_, cy_ = g[:, 8 * n:9 * n], g[:, 6 * n:7 * n]

                cs1 = T("cs1", bufs=early)
                cs2 = tmp_pool.tile([P, 2 * n], dt, tag=f"cs2_{side}",
                                    name=f"cs2_{side}", bufs=early)
                n1 = tmp_pool.tile([P, 3 * n], dt, tag=f"n1_{side}",
                                   name=f"n1_{side}", bufs=early)
                n2 = tmp_pool.tile([P, 3 * n], dt, tag=f"n2_{side}",
                                   name=f"n2_{side}", bufs=early)

                def crossp(o, P01, P12, Q01, Q12, px, py, qx, qy):
                    tt(o[:, 0:2 * n], P01, Q12, Alu.mult)
                    tt(cs2[:], P12, Q01, Alu.mult)
                    tt(o[:, 0:2 * n], o[:, 0:2 * n], cs2[:], Alu.subtract)
                    tt(o[:, 2 * n:3 * n], px, qy, Alu.mult)
                    tt(cs1[:], py, qx, Alu.mult)
                    tt(o[:, 2 * n:3 * n], o[:, 2 * n:3 * n], cs1[:],
                       Alu.subtract)

                crossp(n1, A01, A12, B01, B12, ax_, ay_, bx_, by_)
                crossp(n2, B01, B12, C01, C12, bx_, by_, cx_, cy_)

                # x = n1.n2, D = a.n2 via paired products + half adds
                x = T("x"); D = T("D")
                tt(cs2[:], n1[:, 0:2 * n], n2[:, 0:2 * n], Alu.mult)
                tt(x[:], cs2[:, 0:n], cs2[:, n:2 * n], Alu.add)
                tt(cs1[:], n1[:, 2 * n:3 * n], n2[:, 2 * n:3 * n], Alu.mult)
                tt(x[:], x[:], cs1[:], Alu.add)
                tt(cs2[:], A01, n2[:, n:3 * n], Alu.mult)
                tt(D[:], cs2[:, 0:n], cs2[:, n:2 * n], Alu.add)
                tt(cs1[:], ax_, n2[:, 0:n], Alu.mult)
                tt(D[:], D[:], cs1[:], Alu.add)

                Y = T("Y"); tt(Y[:], D[:], fld(9), Alu.mult)   # D*|dr2|

                xx = T("xx"); nc.scalar.square(xx[:], x[:])
                YY = T("YY"); nc.scalar.square(YY[:], Y[:])
                q = T("q"); tt(q[:], xx[:], YY[:], Alu.add)
                r = T("r", f32); nc.scalar.sqrt(r[:], q[:])
                uf = T("uf", f32)
                nc.vector.reciprocal_approx_fast(uf[:], r[:])
                u = T("u"); nc.scalar.copy(u[:], uf[:])

                # normalized X = cos(phi), Yh = -sin(phi); X^2 + Yh^2 = 1.
                # trig tile layout matches the coef plane order A1 A2 A3 B1
                # B2 B3, so all six Fourier term products collapse into one
                # wide DVE multiply; their sum is absorbed into the PSUM
                # accumulation of the block-sum matmuls.
                Act_ = mybir.ActivationFunctionType
                trig = tmp_pool.tile([P, 6 * n], dt, tag=f"trig_{side}",
                                     name=f"trig_{side}")
                X = trig[:, 0:n]
                Yh = trig[:, n:2 * n]
                c2 = trig[:, 2 * n:3 * n]
                s2 = trig[:, 3 * n:4 * n]
                c3s3 = trig[:, 4 * n:6 * n]
                tt(X, x[:], u[:], Alu.mult)
                tt(Yh, Y[:], u[:], Alu.mult)
                YY1 = T("YY1"); nc.scalar.square(YY1[:], Yh)
                nc.scalar.activation(c2, YY1[:], Act_.Copy, bias=1.0,
                                     scale=-2.0)
                # cc = [c3a|s3a] = [1-4*Yh^2 | 3-4*Yh^2], adjacent so
                # (c3, s3) = cc * (X, Yh) is one double-width multiply
                cc = tmp_pool.tile([P, 2 * n], dt, tag=f"cc_{side}",
                                   name=f"cc_{side}")
                nc.scalar.activation(cc[:, 0:n], YY1[:], Act_.Copy, bias=1.0,
                                     scale=-4.0)
                nc.scalar.activation(cc[:, n:2 * n], YY1[:], Act_.Copy,
                                     bias=3.0, scale=-4.0)
                tt(s2, X, Yh, Alu.mult)
                tt(c3s3, cc[:], trig[:, 0:2 * n], Alu.mult)

                terms = tmp_pool.tile([P, 6 * n], dt, tag=f"terms_{side}",
                                      name=f"terms_{side}")
                for j in range(3):
                    tt(terms[:, 2 * j * n:(2 * j + 2) * n],
                       g[:, (10 + 2 * j) * n:(12 + 2 * j) * n],
                       trig[:, 2 * j * n:(2 * j + 2) * n], Alu.mult)
                return terms

            def do_tile(off, Ft):
                g = gat_pool.tile([P, NPLANES * Ft], dt, tag="g")
                g3 = g[:].rearrange("p (k c) -> p k c", c=Ft)
                for k0, k1 in [(0, 6), (6, 10), (10, NPLANES)]:
                    nc.sync.dma_start(out=g3[:, k0:k1, :],
                                      in_=tbl3[:, k0:k1, off:off + Ft])
                terms = do_side(g[:], "d", Ft)
                pt = ps_pool.tile([NGRP, Ft], f32, tag="ps", name="ps")
                for c0 in range(0, Ft, 512):
                    c1 = min(c0 + 512, Ft)
                    for i in range(6):
                        nc.tensor.matmul(out=pt[:, c0:c1], lhsT=grp[:],
                                         rhs=terms[:, i * Ft + c0:i * Ft + c1],
                                         start=(i == 0), stop=(i == 5))
                nc.scalar.activation(bs[:, off:off + Ft], pt[:], Act.Copy)

            def body():
                off = 0
                for Ft in widths:
                    do_tile(off, Ft)
                    off += Ft

            if repeat > 1:
                with tc.For_i(0, repeat, 1):
                    body()
            else:
                body()

            nc.sync.dma_start(out=out[:], in_=bs[:])
    nc.compile()
    return nc


# --------------------------------------------------------------------------
# end to end
# --------------------------------------------------------------------------

def prepare(inputs, F=1024, F0=384, dtype="bf16"):
    pos = np.asarray(inputs["pos"], dtype=np.float32)
    ty = np.asarray(inputs["atom_types"])
    mapping = np.asarray(inputs["mapping"])
    batch = np.asarray(inputs["mapping_batch"]).astype(np.int64)
    base = np.asarray(mapping[0]).astype(np.int64)
    assert all(np.array_equal(np.asarray(mapping[j]), base + j)
               for j in range(1, 4)), "mapping not consecutive; fast path invalid"
    if np.any(np.diff(batch) < 0):
        # plan_streams assumes batch-sorted dihedrals; energy is invariant
        # to the within-bin order, so a stable sort is safe.
        order = np.argsort(batch, kind="stable")
        base = base[order]
        batch = batch[order]
    n_win = pos.shape[0] - 3
    fields, Cw = build_window_fields(pos, ty, np.asarray(inputs["thetas"]),
                                     np.asarray(inputs["ks"]))
    streams, blk_labels, widths = plan_streams(base, batch, n_win, F, F0)
    np_dtype = ml_dtypes.bfloat16 if dtype == "bf16" else np.float32
    tables = build_core_tables(fields, streams, widths, np_dtype)
    # angle-independent sum_k term, accumulated host-side
    energy_C = np.bincount(batch, weights=Cw[base].astype(np.float64),
                           minlength=NB)
    plan = dict(widths=widths, blk_labels=blk_labels, dtype=dtype,
                energy_C=energy_C)
    return plan, tables


def finish(plan, outs, n_batch=NB):
    """outs: list per core of [NGRP, NCOLS] block sums -> [256] energy."""
    energy = plan["energy_C"].copy()
    for c in range(NCORES):
        bsums = np.asarray(outs[c])          # [NGRP, NCOLS]
        lab = plan["blk_labels"][c]          # [NCOLS*NGRP], -1 = padding
        vals = bsums.T.ravel()               # block (col, grp) order
        m = lab >= 0
        energy += np.bincount(lab[m], weights=vals[m].astype(np.float64),
                              minlength=n_batch)
    return energy.astype(np.float32)


def _kernel_numpy_fallback(pos, atom_types, mapping, mapping_batch, thetas, ks):
    # Correctness safety net for non-consecutive mappings (never expected).
    p0, p1 = pos[mapping[0]], pos[mapping[1]]
    p2, p3 = pos[mapping[2]], pos[mapping[3]]
    dr1, dr2, dr3 = p1 - p0, p2 - p1, p3 - p2
    n1 = np.cross(dr1, dr2); n2 = np.cross(dr2, dr3)
    m1 = np.cross(n1, dr2 / np.linalg.norm(dr2, axis=-1, keepdims=True))
    x = np.sum(n1 * n2, -1); y = np.sum(m1 * n2, -1)
    theta = np.arctan2(y, x)
    t0, t1, t2, t3 = (atom_types[mapping[j]] for j in range(4))
    th = thetas[:, t0, t1, t2, t3]; kk = ks[:, t0, t1, t2, t3]
    degs = np.arange(1, 4)[:, None]
    V = np.sum(kk * (1.0 - np.cos(degs * theta[None, :] - th)), axis=0)
    return np.bincount(mapping_batch, weights=V.astype(np.float64),
                       minlength=256).astype(np.float32)


def kernel(pos, atom_types, mapping, mapping_batch, thetas, ks):
    from concourse.bass_utils import run_bass_kernel_spmd
    pos = np.asarray(pos, dtype=np.float32)
    atom_types = np.asarray(atom_types)
    mapping = np.asarray(mapping)
    mapping_batch = np.asarray(mapping_batch)
    thetas = np.asarray(thetas, dtype=np.float32)
    ks = np.asarray(ks, dtype=np.float32)

    base = np.asarray(mapping[0]).astype(np.int64)
    if not all(np.array_equal(np.asarray(mapping[j]), base + j)
               for j in range(1, 4)):
        print("kernel.py: non-consecutive mapping; numpy fallback",
              file=sys.stderr)
        return _kernel_numpy_fallback(pos, atom_types, mapping, mapping_batch,
                                      thetas, ks)

    inputs = dict(pos=pos, atom_types=atom_types, mapping=mapping,
                  mapping_batch=mapping_batch, thetas=thetas, ks=ks)
    plan, tables = prepare(inputs, F=1024, F0=384, dtype="bf16")
    nc = build_program(plan["widths"], repeat=1, dtype=plan["dtype"])
    in_maps = [{"tbl": tables[c]} for c in range(NCORES)]
    res = run_bass_kernel_spmd(nc, in_maps, list(range(NCORES)))
    outs = [res.results[c]["out"] for c in range(NCORES)]
    return finish(plan, outs).astype(np.float32)
